# revision 11
# baseline (speedup 1.0000x reference)
"""DifferentialDropout Trainium2 kernel (8-core SPMD).

Reference semantics (see problem): per-row corrcoef factor, global-standardized
1000-bin per-row histograms -> entropies -> per-row keep prob -> mask+scale.

Sharding:
  Phase A (D-shard): each core takes a 2048-col slice of temp [1024, 16384]:
    partial row sums / global sum/sumsq/min/max (AllGather + local reduce),
    partial cov = xs @ xs.T via PE (K-sharded GEMM) -> AllReduce (overlapped
    with phase B histogram). Centering applied post-AR as rank-1 correction:
    cov = xxT - outer(rowsum, rowsum)/D.
  Phase B (B-shard): each core owns 128 rows: exact per-row 1000-bin histogram
    (radix 32x32: bf16 one-hot planes on DVE, combined per 128-element k-tile
    by PE matmuls accumulating [32q, 32l] counts in PSUM), entropies, factors,
    and the final mask/scale pass.
"""

import sys

sys.path.insert(0, "/opt/trn_rl_repo")

import numpy as np

B = 1024
D = 16384
BINS = 1000
N_CORES = 8
DSL = D // N_CORES      # 2048
RSL = B // N_CORES      # 128
C_ROWS = 2              # rows per histogram chunk
F = C_ROWS * 128        # free elems/partition/chunk
N_CH = RSL // C_ROWS
LN2 = 0.6931471805599453

_cache = {}


def _build():
    import concourse.mybir as mybir
    import concourse.tile as tile
    from concourse import bacc
    from concourse.masks import make_identity

    F32 = mybir.dt.float32
    BF16 = mybir.dt.bfloat16
    A = mybir.AluOpType
    AF = mybir.ActivationFunctionType
    AX = mybir.AxisListType.X

    nc = bacc.Bacc("TRN2", target_bir_lowering=False, debug=False,
                   num_devices=N_CORES)

    xs = nc.dram_tensor("xs", [B, DSL], F32, kind="ExternalInput")
    xr = nc.dram_tensor("xr", [RSL, D], F32, kind="ExternalInput")
    ur = nc.dram_tensor("ur", [RSL, D], F32, kind="ExternalInput")
    sel = nc.dram_tensor("sel", [128, 8], F32, kind="ExternalInput")
    out = nc.dram_tensor("out", [RSL, D], F32, kind="ExternalOutput")
    dbg = nc.dram_tensor("dbg", [128, 16], F32, kind="ExternalOutput")
    dbg2 = nc.dram_tensor("dbg2", [128, 48], F32, kind="ExternalOutput")
    dbg3 = nc.dram_tensor("dbg3", [128, 12], F32, kind="ExternalOutput")

    xr_v = xr.ap().rearrange("r (p e) -> p r e", p=128)   # [128, 128, 128]

    with tile.TileContext(nc) as tc:
        with (
            tc.tile_pool(name="const", bufs=1) as constp,
            tc.tile_pool(name="persist", bufs=1) as persist,
            tc.tile_pool(name="dram", bufs=1, space="DRAM") as dram,
        ):
            id128 = constp.tile([128, 128], F32, name="id128")
            make_identity(nc, id128[:])
            ones32 = constp.tile([32, 1], F32, name="ones32")
            nc.vector.memset(ones32[:], 1.0)
            epsb = constp.tile([128, 1], F32, name="epsb")
            nc.vector.memset(epsb[:], 1e-30)

            # persistent SBUF
            counts_sb = persist.tile([32, RSL * 32], F32, name="counts_sb")
            scal = persist.tile([128, 24], F32, name="scal")  # scalar consts
            ag_sb = persist.tile([128, 18], F32, name="ag_sb")
            agg_sb = persist.tile([128, 8, 18], F32, name="agg_sb")
            rowsum = persist.tile([128, 8], F32, name="rowsum")
            rsDn = persist.tile([128, 8], F32, name="rsDn")
            rsb = persist.tile([128, 1024], F32, name="rsb")
            rdb = persist.tile([128, 1024], F32, name="rdb")
            dcol = persist.tile([128, 8], F32, name="dcol")
            f1col = persist.tile([128, 8], F32, name="f1col")
            sel_sb = persist.tile([128, 8], F32, name="sel_sb")
            pvec = persist.tile([128, 4], F32, name="pvec")  # p, rkeep, keep, f1own

            nc.sync.dma_start(sel_sb[:], sel.ap())

            # DRAM bounces
            ag_in = dram.tile([128, 18], F32, name="ag_in")
            ag_out = dram.tile([1024, 18], F32, addr_space="Shared", name="ag_out")
            cov_in = dram.tile([1024, 1024], F32, name="cov_in")
            cov_out = dram.tile([1024, 1024], F32, addr_space="Shared", name="cov_out")
            bc_in = dram.tile([32, 32], F32, name="bc_in")
            bc_out = dram.tile([32, 32], F32, addr_space="Shared", name="bc_out")

            # ---------------- Phase A: stats + transposes + GEMM ------------
            with (
                tc.tile_pool(name="pa_io", bufs=3) as pa_io,
                tc.tile_pool(name="pa_big", bufs=1) as pa_big,
                tc.tile_pool(name="pa_ps", bufs=2, space="PSUM") as pa_ps,
                tc.tile_pool(name="pa_ps2", bufs=2, space="PSUM") as pa_ps2,
                tc.tile_pool(name="pa_w", bufs=2) as pa_w,
            ):
                xsT = pa_big.tile([128, 16, 1024], F32, name="xsT")
                covp = pa_big.tile([128, 8, 1024], F32, name="covp")
                mn_col = pa_w.tile([128, 8], F32, name="mn_col", bufs=1)
                mx_col = pa_w.tile([128, 8], F32, name="mx_col", bufs=1)

                for t in range(8):
                    xa = pa_io.tile([128, DSL], F32, name="xa")
                    nc.sync.dma_start(xa[:], xs.ap()[t * 128:(t + 1) * 128, :])
                    nc.vector.reduce_sum(ag_sb[:, t:t + 1], xa[:], axis=AX)
                    sq_scr = pa_w.tile([128, DSL], F32, name="sq_scr")
                    nc.scalar.activation(sq_scr[:], xa[:], AF.Square,
                                         accum_out=ag_sb[:, 8 + t:9 + t])
                    nc.vector.reduce_sum(mn_col[:, t:t + 1], xa[:], axis=AX, op=A.min)
                    nc.vector.reduce_sum(mx_col[:, t:t + 1], xa[:], axis=AX, op=A.max)
                    for k in range(16):
                        pt = pa_ps.tile([128, 128], F32, name="pt")
                        nc.tensor.transpose(pt[:], xa[:, k * 128:(k + 1) * 128], id128[:])
                        nc.scalar.copy(xsT[:, k, t * 128:(t + 1) * 128], pt[:])

                nc.vector.reduce_sum(ag_sb[:, 16:17], mn_col[:], axis=AX, op=A.min)
                nc.vector.reduce_sum(ag_sb[:, 17:18], mx_col[:], axis=AX, op=A.max)
                nc.sync.dma_start(ag_in[:], ag_sb[:])
                nc.gpsimd.collective_compute(
                    "AllGather", A.bypass,
                    replica_groups=[list(range(N_CORES))],
                    ins=[ag_in.opt()], outs=[ag_out.opt()])
                nc.sync.dma_start(
                    agg_sb[:], ag_out[:].rearrange("(r p) c -> p r c", p=128))

                # GEMM: covp[m, :] = sum_k xsT_k[:, m-block].T @ xsT_k
                for m in range(8):
                    for n2 in range(2):
                        pg = pa_ps2.tile([128, 512], F32, name="pg")
                        for k in range(16):
                            nc.tensor.matmul(
                                pg[:],
                                xsT[:, k, m * 128:(m + 1) * 128],
                                xsT[:, k, n2 * 512:(n2 + 1) * 512],
                                start=(k == 0), stop=(k == 15))
                        nc.scalar.copy(covp[:, m, n2 * 512:(n2 + 1) * 512], pg[:])
                nc.sync.dma_start(
                    cov_in[:].rearrange("(m p) j -> p m j", p=128), covp[:])
                nc.gpsimd.collective_compute(
                    "AllReduce", A.add,
                    replica_groups=[list(range(N_CORES))],
                    ins=[cov_in.opt()], outs=[cov_out.opt()])

                # -------- stats reduction + scalar constants --------
                # rowsum[p, t] = sum over ranks
                nc.vector.reduce_sum(
                    rowsum[:], agg_sb[:, :, 0:8].rearrange("p r c -> p c r"), axis=AX)
                sq8 = pa_w.tile([128, 8], F32, name="sq8", bufs=1)
                nc.vector.reduce_sum(
                    sq8[:], agg_sb[:, :, 8:16].rearrange("p r c -> p c r"), axis=AX)
                # scal columns: 0 gsum, 1 gss, 2 negmn, 3 gmx, 4 mu, 5 sd, 6 rsd,
                # 7 lo, 8 width, 9 rwidth, 10 SC, 11 BC, 12 rnw_l, 13 rnw_b,
                # 14 cH_l, 15 cH_b, 16..23 scratch
                nc.vector.reduce_sum(scal[:, 16:17], rowsum[:], axis=AX)
                nc.gpsimd.partition_all_reduce(scal[:, 0:1], scal[:, 16:17], 128,
                                               _reduce_add())
                nc.vector.reduce_sum(scal[:, 17:18], sq8[:], axis=AX)
                nc.gpsimd.partition_all_reduce(scal[:, 1:2], scal[:, 17:18], 128,
                                               _reduce_add())
                nc.vector.reduce_sum(
                    scal[:, 18:19],
                    agg_sb[:, :, 16:17].rearrange("p r o -> p (r o)"), axis=AX, op=A.min)
                nc.vector.tensor_single_scalar(scal[:, 18:19], scal[:, 18:19], -1.0, A.mult)
                nc.gpsimd.partition_all_reduce(scal[:, 2:3], scal[:, 18:19], 128,
                                               _reduce_max())
                nc.vector.reduce_sum(
                    scal[:, 19:20],
                    agg_sb[:, :, 17:18].rearrange("p r o -> p (r o)"), axis=AX, op=A.max)
                nc.gpsimd.partition_all_reduce(scal[:, 3:4], scal[:, 19:20], 128,
                                               _reduce_max())

                N_f = float(B) * float(D)
                # mu = gsum/N
                nc.vector.tensor_single_scalar(scal[:, 4:5], scal[:, 0:1], 1.0 / N_f, A.mult)
                # var = (gss - gsum*mu)/(N-1); sd = sqrt
                nc.vector.tensor_mul(scal[:, 20:21], scal[:, 0:1], scal[:, 4:5])
                nc.vector.tensor_sub(scal[:, 20:21], scal[:, 1:2], scal[:, 20:21])
                nc.vector.tensor_single_scalar(scal[:, 20:21], scal[:, 20:21],
                                               1.0 / (N_f - 1.0), A.mult)
                nc.scalar.activation(scal[:, 5:6], scal[:, 20:21], AF.Sqrt)
                nc.vector.reciprocal(scal[:, 6:7], scal[:, 5:6])
                # lo = (tmin - mu)*rsd ; tmin = -negmn
                nc.vector.tensor_single_scalar(scal[:, 21:22], scal[:, 2:3], -1.0, A.mult)
                nc.vector.tensor_sub(scal[:, 21:22], scal[:, 21:22], scal[:, 4:5])
                nc.vector.tensor_mul(scal[:, 7:8], scal[:, 21:22], scal[:, 6:7])
                # hi = (tmax - mu)*rsd ; width = (hi - lo)/BINS
                nc.vector.tensor_sub(scal[:, 22:23], scal[:, 3:4], scal[:, 4:5])
                nc.vector.tensor_mul(scal[:, 22:23], scal[:, 22:23], scal[:, 6:7])
                nc.vector.tensor_sub(scal[:, 22:23], scal[:, 22:23], scal[:, 7:8])
                nc.vector.tensor_single_scalar(scal[:, 8:9], scal[:, 22:23],
                                               1.0 / BINS, A.mult)
                nc.vector.reciprocal(scal[:, 9:10], scal[:, 8:9])
                # SC = rsd*rwidth ; BC = -(mu*rsd + lo)*rwidth
                nc.vector.tensor_mul(scal[:, 10:11], scal[:, 6:7], scal[:, 9:10])
                nc.vector.tensor_mul(scal[:, 23:24], scal[:, 4:5], scal[:, 6:7])
                nc.vector.tensor_add(scal[:, 23:24], scal[:, 23:24], scal[:, 7:8])
                nc.vector.tensor_mul(scal[:, 23:24], scal[:, 23:24], scal[:, 9:10])
                nc.vector.tensor_single_scalar(scal[:, 11:12], scal[:, 23:24], -1.0, A.mult)
                # entropy consts: rnw_l = 1/(width*D), rnw_b = 1/(width*N)
                nc.vector.tensor_single_scalar(scal[:, 16:17], scal[:, 8:9], float(D), A.mult)
                nc.vector.reciprocal(scal[:, 12:13], scal[:, 16:17])
                nc.vector.tensor_single_scalar(scal[:, 17:18], scal[:, 8:9], N_f, A.mult)
                nc.vector.reciprocal(scal[:, 13:14], scal[:, 17:18])
                nc.vector.tensor_single_scalar(scal[:, 14:15], scal[:, 12:13],
                                               -1.0 / LN2, A.mult)
                nc.vector.tensor_single_scalar(scal[:, 15:16], scal[:, 13:14],
                                               -1.0 / LN2, A.mult)
                # rsDn = rowsum * (-1/D)
                nc.vector.tensor_single_scalar(rsDn[:], rowsum[:], -1.0 / float(D), A.mult)
                # rsb = broadcast of flattened rowsum
                _bcast_cols(nc, pa_w, pa_ps, rowsum, rsb, id128)

            # ---------------- Phase B: histogram ----------------------------
            with (
                tc.tile_pool(name="hb_io", bufs=3) as hb_io,
                tc.tile_pool(name="hb_w", bufs=2) as hb_w,
                tc.tile_pool(name="hb_pl", bufs=2) as hb_pl,
                tc.tile_pool(name="hb_ps", bufs=4, space="PSUM") as hb_ps,
            ):
                for ch in range(N_CH):
                    r0 = ch * C_ROWS
                    xch = hb_io.tile([128, C_ROWS, 128], F32, name="xch")
                    nc.sync.dma_start(xch[:], xr_v[:, r0:r0 + C_ROWS, :])
                    xf = xch[:].rearrange("p a b -> p (a b)")

                    v = hb_w.tile([128, F], F32, name="v")
                    nc.scalar.activation(v[:], xf, AF.Identity,
                                         bias=scal[:, 11:12], scale=scal[:, 10:11])
                    i1 = hb_w.tile([128, F], mybir.dt.int32, name="i1")
                    nc.vector.tensor_copy(i1[:], v[:])
                    f1t = hb_w.tile([128, F], F32, name="f1t")
                    nc.vector.tensor_copy(f1t[:], i1[:])
                    g1 = hb_w.tile([128, F], F32, name="g1")
                    nc.vector.tensor_tensor(g1[:], f1t[:], v[:], A.is_gt)
                    idx = hb_w.tile([128, F], F32, name="idx")
                    nc.vector.tensor_sub(idx[:], f1t[:], g1[:])
                    nc.vector.tensor_scalar(idx[:], idx[:], 999.0, 0.0, A.min, A.max)

                    s2 = hb_w.tile([128, F], F32, name="s2")
                    nc.vector.tensor_single_scalar(s2[:], idx[:], 1.0 / 32.0, A.mult)
                    i2 = hb_w.tile([128, F], mybir.dt.int32, name="i2")
                    nc.vector.tensor_copy(i2[:], s2[:])
                    f2t = hb_w.tile([128, F], F32, name="f2t")
                    nc.vector.tensor_copy(f2t[:], i2[:])
                    g2 = hb_w.tile([128, F], F32, name="g2")
                    nc.vector.tensor_tensor(g2[:], f2t[:], s2[:], A.is_gt)
                    hi_f = hb_w.tile([128, F], F32, name="hi_f")
                    nc.vector.tensor_sub(hi_f[:], f2t[:], g2[:])
                    lo_f = hb_w.tile([128, F], F32, name="lo_f")
                    nc.vector.scalar_tensor_tensor(lo_f[:], hi_f[:], -32.0, idx[:],
                                                   A.mult, A.add)
                    hi_b = hb_w.tile([128, F], BF16, name="hi_b")
                    nc.vector.tensor_copy(hi_b[:], hi_f[:])
                    lo_b = hb_w.tile([128, F], BF16, name="lo_b")
                    nc.vector.tensor_copy(lo_b[:], lo_f[:])

                    Hpl = hb_pl.tile([128, 32 * F], BF16, name="Hpl")
                    Lpl = hb_pl.tile([128, 32 * F], BF16, name="Lpl")
                    for q in range(32):
                        nc.vector.tensor_single_scalar(
                            Hpl[:, q * F:(q + 1) * F], hi_b[:], float(q), A.is_equal)
                        nc.vector.tensor_single_scalar(
                            Lpl[:, q * F:(q + 1) * F], lo_b[:], float(q), A.is_equal)

                    Hv = Hpl[:].rearrange("p (q f) -> p f q", q=32)
                    Lv = Lpl[:].rearrange("p (q f) -> p f q", q=32)
                    for r in range(C_ROWS):
                        ps = hb_ps.tile([32, 32], F32, name="ps")
                        for k in range(128):
                            t = r * 128 + k
                            nc.tensor.matmul(ps[:], Hv[:, t, :], Lv[:, t, :],
                                             start=(k == 0), stop=(k == 127))
                        rr = r0 + r
                        nc.scalar.copy(counts_sb[:, rr * 32:(rr + 1) * 32], ps[:32, :])

                # batch-count partial + AllReduce
                bc_sb = hb_w.tile([32, 32], F32, name="bc_sb", bufs=1)
                nc.vector.reduce_sum(
                    bc_sb[:],
                    counts_sb[:].rearrange("p (r l) -> p l r", r=RSL), axis=AX)
                nc.sync.dma_start(bc_in[:], bc_sb[:])
                nc.gpsimd.collective_compute(
                    "AllReduce", A.add,
                    replica_groups=[list(range(N_CORES))],
                    ins=[bc_in.opt()], outs=[bc_out.opt()])

            # ---------------- Phase C: factors + entropies + mask -----------
            with (
                tc.tile_pool(name="pc_w", bufs=2) as pc_w,
                tc.tile_pool(name="pc_big", bufs=1) as pc_big,
                tc.tile_pool(name="pc_ps", bufs=2, space="PSUM") as pc_ps,
                tc.tile_pool(name="pc_io", bufs=3) as pc_io,
            ):
                covc = pc_big.tile([128, 8, 1024], F32, name="covc")
                # load AR'd cov, apply rank-1 centering, extract diag
                for m in range(8):
                    nc.sync.dma_start(
                        covc[:, m, :], cov_out[:][m * 128:(m + 1) * 128, :])
                    nc.vector.scalar_tensor_tensor(
                        covc[:, m, :], rsb[:], rsDn[:, m:m + 1], covc[:, m, :],
                        A.mult, A.add)
                    dtmp = pc_w.tile([128, 128], F32, name="dtmp")
                    nc.vector.tensor_mul(dtmp[:], covc[:, m, m * 128:(m + 1) * 128],
                                         id128[:])
                    nc.vector.reduce_sum(dcol[:, m:m + 1], dtmp[:], axis=AX)
                nc.scalar.activation(dcol[:], dcol[:], AF.Sqrt)
                rdc = pc_w.tile([128, 8], F32, name="rdc", bufs=1)
                nc.vector.reciprocal(rdc[:], dcol[:])
                _bcast_cols(nc, pc_w, pc_ps, rdc, rdb, id128)
                dbg3_sb = pc_w.tile([128, 12], F32, name="dbg3_sb", bufs=1)
                for m in range(8):
                    t1 = pc_w.tile([128, 1024], F32, name="t1")
                    nc.vector.tensor_mul(t1[:], covc[:, m, :], rdb[:])
                    t1b = pc_w.tile([128, 1024], F32, name="t1b")
                    nc.vector.tensor_single_scalar(t1b[:], t1[:], rdc[:, m:m + 1], A.mult)
                    t1c = pc_w.tile([128, 1024], F32, name="t1c")
                    nc.scalar.activation(t1c[:], t1b[:], AF.Abs,
                                         accum_out=f1col[:, m:m + 1])
                    if m == 0:
                        nc.scalar.copy(dbg3_sb[:, 0:4], t1[:, 0:4])
                        nc.scalar.copy(dbg3_sb[:, 4:8], t1b[:, 0:4])
                        nc.scalar.copy(dbg3_sb[:, 8:12], rdc[:, 0:4])
                # f1_own = sel . f1col / B
                f1s = pc_w.tile([128, 8], F32, name="f1s", bufs=1)
                nc.vector.tensor_mul(f1s[:], f1col[:], sel_sb[:])
                nc.vector.reduce_sum(pvec[:, 3:4], f1s[:], axis=AX)
                nc.vector.tensor_single_scalar(pvec[:, 3:4], pvec[:, 3:4],
                                               1.0 / float(B), A.mult)

                # local entropies
                lnch = pc_big.tile([32, RSL * 32], F32, name="lnch")
                nc.scalar.activation(lnch[:], counts_sb[:], AF.Ln,
                                     scale=scal[0:32, 12:13], bias=epsb[0:32, :])
                nc.vector.tensor_mul(lnch[:], lnch[:], counts_sb[:])
                erp = pc_w.tile([32, RSL], F32, name="erp", bufs=1)
                nc.vector.reduce_sum(
                    erp[:], lnch[:].rearrange("p (r l) -> p r l", r=RSL), axis=AX)
                psS = pc_ps.tile([1, RSL], F32, name="psS")
                nc.tensor.matmul(psS[:], ones32[:], erp[:], start=True, stop=True)
                srow = pc_w.tile([1, RSL], F32, name="srow", bufs=1)
                nc.scalar.copy(srow[:], psS[:])
                psT = pc_ps.tile([128, 1], F32, name="psT")
                nc.tensor.transpose(psT[:], srow[:], id128[:1, :1])
                hloc = pc_w.tile([128, 1], F32, name="hloc", bufs=1)
                nc.scalar.copy(hloc[:], psT[:])
                nc.vector.tensor_mul(hloc[:], hloc[:], scal[:, 14:15])

                # batch entropy
                bcs = pc_w.tile([32, 32], F32, name="bcs", bufs=1)
                nc.sync.dma_start(bcs[:], bc_out[:])
                lnb = pc_w.tile([32, 32], F32, name="lnb", bufs=1)
                nc.scalar.activation(lnb[:], bcs[:], AF.Ln,
                                     scale=scal[0:32, 13:14], bias=epsb[0:32, :])
                nc.vector.tensor_mul(lnb[:], lnb[:], bcs[:])
                sb1 = pc_w.tile([32, 1], F32, name="sb1", bufs=1)
                nc.vector.reduce_sum(sb1[:], lnb[:], axis=AX)
                nc.gpsimd.partition_all_reduce(sb1[:], sb1[:], 32, _reduce_add())
                hbat = pc_w.tile([128, 1], F32, name="hbat", bufs=1)
                nc.gpsimd.partition_broadcast(hbat[:], sb1[0:1, :])
                nc.vector.tensor_mul(hbat[:], hbat[:], scal[:, 15:16])

                # f2' = max(f2, 1/f2); keep = f1/f2'; p = 1-keep; rkeep = 1/keep
                tA = pc_w.tile([128, 1], F32, name="tA", bufs=1)
                tB = pc_w.tile([128, 1], F32, name="tB", bufs=1)
                nc.vector.reciprocal(tA[:], hbat[:])
                f2 = pc_w.tile([128, 1], F32, name="f2", bufs=1)
                nc.vector.tensor_mul(f2[:], hloc[:], tA[:])
                nc.vector.reciprocal(tB[:], f2[:])
                nc.vector.tensor_max(f2[:], f2[:], tB[:])
                nc.vector.reciprocal(tB[:], f2[:])
                nc.vector.tensor_mul(pvec[:, 2:3], pvec[:, 3:4], tB[:])
                nc.vector.tensor_scalar(pvec[:, 0:1], pvec[:, 2:3], -1.0, 1.0,
                                        A.mult, A.add)
                nc.vector.reciprocal(pvec[:, 1:2], pvec[:, 2:3])

                dbg_sb = pc_w.tile([128, 16], F32, name="dbg_sb", bufs=1)
                nc.scalar.copy(dbg_sb[:, 0:4], pvec[:])
                nc.scalar.copy(dbg_sb[:, 4:5], hloc[:])
                nc.scalar.copy(dbg_sb[:, 5:6], hbat[:])
                nc.scalar.copy(dbg_sb[:, 6:14], scal[:, 4:12])
                nc.scalar.copy(dbg_sb[:, 14:16], scal[:, 12:14])
                nc.sync.dma_start(dbg.ap(), dbg_sb[:])
                dbg2_sb = pc_w.tile([128, 48], F32, name="dbg2_sb", bufs=1)
                nc.scalar.copy(dbg2_sb[:, 0:8], dcol[:])
                nc.scalar.copy(dbg2_sb[:, 8:16], f1col[:])
                nc.scalar.copy(dbg2_sb[:, 16:24], rsb[:, 0:8])
                nc.scalar.copy(dbg2_sb[:, 24:32], rdb[:, 0:8])
                nc.scalar.copy(dbg2_sb[:, 32:40], covc[:, 0, 0:8])
                nc.scalar.copy(dbg2_sb[:, 40:48], rowsum[:])
                nc.sync.dma_start(dbg2.ap(), dbg2_sb[:])
                nc.sync.dma_start(dbg3.ap(), dbg3_sb[:])

                # mask + scale
                CH = 2048
                for c in range(D // CH):
                    xm = pc_io.tile([128, CH], F32, name="xm")
                    um = pc_io.tile([128, CH], F32, name="um")
                    nc.sync.dma_start(xm[:], xr.ap()[:, c * CH:(c + 1) * CH])
                    nc.sync.dma_start(um[:], ur.ap()[:, c * CH:(c + 1) * CH])
                    msk = pc_io.tile([128, CH], F32, name="msk")
                    nc.vector.tensor_single_scalar(msk[:], um[:], pvec[:, 0:1], A.is_gt)
                    oc = pc_io.tile([128, CH], F32, name="oc")
                    nc.vector.scalar_tensor_tensor(oc[:], msk[:], pvec[:, 1:2], xm[:],
                                                   A.mult, A.mult)
                    nc.sync.dma_start(out.ap()[:, c * CH:(c + 1) * CH], oc[:])

    nc.compile()
    return nc


def _reduce_add():
    from concourse import bass_isa
    return bass_isa.ReduceOp.add


def _reduce_max():
    from concourse import bass_isa
    return bass_isa.ReduceOp.max


def _bcast_cols(nc, sbuf_pool, psum_pool, vec8, dst, id128):
    """dst[p, t*128+q] = vec8[q, t]  (flatten [128,8] col-major, bcast to all
    partitions)."""
    import concourse.mybir as mybir
    F32 = mybir.dt.float32
    pt = psum_pool.tile([8, 128], F32, name="bc_pt")
    nc.tensor.transpose(pt[:8, :], vec8[:], id128[:])
    tr = sbuf_pool.tile([8, 128], F32, name="bc_tr", bufs=1)
    nc.scalar.copy(tr[:], pt[:8, :])
    flat = sbuf_pool.tile([1, 8 * 128], F32, name="bc_flat", bufs=1)
    for t in range(8):
        nc.sync.dma_start(flat[:, t * 128:(t + 1) * 128], tr[t:t + 1, :])
    nc.gpsimd.partition_broadcast(dst[:], flat[:])


def kernel(x, u):
    if "nc" not in _cache:
        _cache["nc"] = _build()
    nc = _cache["nc"]
    from concourse.bass_utils import run_bass_kernel_spmd

    xf = np.ascontiguousarray(x.reshape(B, D), dtype=np.float32)
    uf = np.ascontiguousarray(u.reshape(B, D), dtype=np.float32)
    in_maps = []
    for c in range(N_CORES):
        selv = np.zeros((128, 8), np.float32)
        selv[:, c] = 1.0
        in_maps.append({
            "xs": np.ascontiguousarray(xf[:, c * DSL:(c + 1) * DSL]),
            "xr": np.ascontiguousarray(xf[c * RSL:(c + 1) * RSL, :]),
            "ur": np.ascontiguousarray(uf[c * RSL:(c + 1) * RSL, :]),
            "sel": selv,
        })
    res = run_bass_kernel_spmd(nc, in_maps, core_ids=list(range(N_CORES)))
    _cache["last_results"] = res
    outf = np.concatenate([res.results[c]["out"] for c in range(N_CORES)], axis=0)
    return outf.reshape(x.shape)
